# revision 14
# baseline (speedup 1.0000x reference)
"""BiChain kernel for 8x TRN2 NeuronCores (data-parallel over batch).

Math: for each chain (fwd, rev), score_i = sigmoid(<[src, s_0..s_{i-1}], w_i> + b_i).
Split w_i into the dense part (first 1024 cols) and the tiny triangular coupling
U[i,j] = W[i, 1024+j].  Then  S = sigmoid(G + b + U S)  with  G = src @ Wd.T,
solved by one Jacobi refinement (U is nilpotent, coupling norm ~0.3):
S1 = sigmoid(G+b), S2 = sigmoid(G+b + U S1).  The rev chain is stored
row-reversed so the final combine 0.5*(S_f + S_r) is row-aligned and fused with
the transpose back to [batch, 40] as a single matmul against [0.5*I; 0.5*I].

Layout: src is cast to fp16 and TRANSPOSED on the host, so the device load is a
fully contiguous [D, batch] read (half the HBM traffic of the f32 source, no
on-chip transposes at all).  G accumulates in PSUM [80, batch-group]; the U*S1
refinement matmul accumulates onto the still-open G group, so G never leaves
PSUM.  Per-256-column pipeline: 8 G matmuls -> sigmoid -> U matmul -> sigmoid
-> fuse/transpose matmul -> store, software-pipelined across batch groups.
"""

import os
import sys

sys.path.insert(0, "/opt/trn_rl_repo")

import numpy as np

B, D, C = 32768, 1024, 40
C2 = 2 * C
N_CORES = 8
BS = B // N_CORES          # 4096 rows per core
P = 128
NKC = D // P               # 8 contraction chunks
BGS = int(os.environ.get("BICHAIN_BGS", "256"))   # batch-group (pipeline step) size
NST = BS // BGS            # pipeline steps per core
TPG = BGS // P             # output row-tiles per step
NT = BS // P               # 32 output row-tiles per core

_CACHE = {}


def _host_prep(W, b, W_rev, b_rev):
    Wr = W_rev[::-1].copy()
    br = b_rev[::-1].copy()
    Uf = np.zeros((C, C), np.float32)
    Ur = np.zeros((C, C), np.float32)
    for i in range(C):
        for j in range(C):
            if j < i:
                Uf[i, j] = W[i, D + j]
            if j > i:
                Ur[i, j] = Wr[i, D + (C - 1 - j)]
    Wd = np.concatenate([W[:, :D], Wr[:, :D]], axis=0)            # [80, 1024]
    # wt[p, kc, m] = Wd.T[kc*128 + p, m]: contiguous per-partition const load
    wt = np.ascontiguousarray(
        Wd.T.reshape(NKC, P, C2).transpose(1, 0, 2)
    ).astype(np.float16)                                          # [128, 8, 80]
    u2t = np.zeros((C2, C2), np.float32)
    u2t[:C, :C] = Uf.T
    u2t[C:, C:] = Ur.T
    u2t = u2t.astype(np.float16)
    bvec = np.concatenate([b, br]).reshape(C2, 1).astype(np.float32)
    halfi = np.zeros((C2, C), np.float32)
    halfi[np.arange(C), np.arange(C)] = 0.5
    halfi[C + np.arange(C), np.arange(C)] = 0.5
    halfi = halfi.astype(np.float16)
    return {"wt": wt.reshape(P, NKC * C2), "u2t": u2t, "bvec": bvec, "halfi": halfi}


def build_nc():
    from concourse import bacc, mybir
    from concourse.tile import TileContext

    dt = mybir.dt
    AF = mybir.ActivationFunctionType

    nc = bacc.Bacc(None, target_bir_lowering=False, debug=False)
    # srcT row g*128+p holds chunk g's data for partition p as one contiguous
    # 4KB run: [kc, n] for n in the chunk's 256 batch columns
    srcT = nc.declare_dram_parameter("srcT", [NST * P, NKC * BGS], dt.float16, isOutput=False)
    wt = nc.declare_dram_parameter("wt", [P, NKC * C2], dt.float16, isOutput=False)
    u2t = nc.declare_dram_parameter("u2t", [C2, C2], dt.float16, isOutput=False)
    bvec = nc.declare_dram_parameter("bvec", [C2, 1], dt.float32, isOutput=False)
    halfi = nc.declare_dram_parameter("halfi", [C2, C], dt.float16, isOutput=False)
    out = nc.declare_dram_parameter("out", [BS, C], dt.float32, isOutput=True)

    with TileContext(nc) as tc:
        with (
            tc.tile_pool(name="const", bufs=1) as cpool,
            tc.tile_pool(name="big", bufs=1) as bigpool,
            tc.tile_pool(name="gps", bufs=3, space="PSUM") as gpool,
            tc.tile_pool(name="ops", bufs=2, space="PSUM") as opool,
        ):
            # consts go via the sync engine's queue so they don't occupy the
            # issue slots of the src stream
            wt_sb = cpool.tile([P, NKC, C2], dt.float16)
            nc.sync.dma_start(
                out=wt_sb[:], in_=wt[:].rearrange("p (kc m) -> p kc m", m=C2)
            )
            u2t_sb = cpool.tile([C2, C2], dt.float16)
            nc.sync.dma_start(out=u2t_sb[:], in_=u2t[:])
            b_sb = cpool.tile([C2, 1], dt.float32)
            nc.sync.dma_start(out=b_sb[:], in_=bvec[:])
            halfi_sb = cpool.tile([C2, C], dt.float16)
            nc.sync.dma_start(out=halfi_sb[:], in_=halfi[:])

            # srcT_sb[p, g, kc, n] = src[g*256 + n, kc*128 + p]
            srcT_sb = bigpool.tile([P, NST, NKC, BGS], dt.float16)
            s1 = bigpool.tile([C2, BS], dt.float16)
            sfin = bigpool.tile([C2, BS], dt.float16)
            outst = bigpool.tile([64, NST, 4, C], dt.float32)

            srcT_pt = srcT[:].rearrange("(g p) x -> p g x", p=P)
            for g in range(NST):
                # one sw ring, strict FIFO: chunks complete in issue order
                nc.gpsimd.dma_start(
                    out=srcT_sb[:, g, :, :].rearrange("p kc n -> p (kc n)"),
                    in_=srcT_pt[:, g, :],
                )

            # batch row = s*BGS + 4p + e  ->  [64, NST, 4*C] with 640B runs
            out_pt = out[:].rearrange("(s p e) c -> p s (e c)", p=64, e=4)
            gtiles = [None] * NST

            # software-pipelined steps: G(s) | jacobi(s-1) | fuse+store(s-2)
            for step in range(NST + 2):
                if step < NST:
                    s = step
                    sl = slice(s * BGS, (s + 1) * BGS)
                    ps = gpool.tile([C2, 512], dt.float32, name="psg")  # full psum bank
                    gtiles[s] = ps
                    for kc in range(NKC):
                        nc.tensor.matmul(
                            ps[:, :BGS],
                            lhsT=wt_sb[:, kc, :],
                            rhs=srcT_sb[:, s, kc, :],
                            start=(kc == 0),
                            stop=False,   # group stays open for the U*S1 accumulate
                        )
                    # S1 = sigmoid(G + b) straight off psum (bias is per-partition)
                    nc.scalar.activation(
                        out=s1[:, sl], in_=ps[:, :BGS], func=AF.Sigmoid, bias=b_sb[:]
                    )
                if 1 <= step <= NST:
                    s = step - 1
                    sl = slice(s * BGS, (s + 1) * BGS)
                    ps = gtiles[s]
                    # G += U @ S1 (accumulates onto the open group; WAR on the
                    # sigmoid read is handled by the tile dep tracker)
                    nc.tensor.matmul(
                        ps[:, :BGS],
                        lhsT=u2t_sb[:],
                        rhs=s1[:, sl],
                        start=False,
                        stop=True,
                    )
                    nc.scalar.activation(
                        out=sfin[:, sl], in_=ps[:, :BGS], func=AF.Sigmoid, bias=b_sb[:]
                    )
                if step >= 2:
                    s = step - 2
                    # fused 0.5*(S_f + S_r) + transpose back to [batch, 40].
                    # Partition p takes batch rows s*BGS + 4p + e, so the store
                    # writes 640B-contiguous runs per partition (fast DMA).
                    sfe = sfin[:, s * BGS : (s + 1) * BGS].rearrange(
                        "c (q e) -> c e q", e=4
                    )
                    ps_o = opool.tile([64, 4, C], dt.float32, name="pso")
                    for e in range(4):
                        nc.tensor.matmul(
                            ps_o[:, e, :],
                            lhsT=sfe[:, e, :],
                            rhs=halfi_sb[:],
                            start=(e == 0),
                            stop=(e == 3),
                        )
                    nc.vector.tensor_copy(outst[:, s, :, :], ps_o[:])
                    # stores ride SP's hw queue (gpsimd's sw ring would FIFO
                    # them behind the whole src stream); batch 2 steps per DMA
                    if s % 2 == 1:
                        nc.sync.dma_start(
                            out=out_pt[:, s - 1 : s + 1, :],
                            in_=outst[:, s - 1 : s + 1, :, :].rearrange(
                                "p s e c -> p s (e c)"
                            ),
                        )

    nc.compile()
    return nc


def _get_nc():
    if "nc" not in _CACHE:
        _CACHE["nc"] = build_nc()
    return _CACHE["nc"]


def _build_in_maps(src, W, b, W_rev, b_rev):
    prep = _host_prep(W, b, W_rev, b_rev)
    src16 = np.asarray(src, dtype=np.float16)
    in_maps = []
    for c in range(N_CORES):
        m = dict(prep)
        # [NST*P, NKC*BGS]: row g*128+p = [kc, n] slab for chunk g, partition p
        blk = src16[c * BS : (c + 1) * BS].reshape(NST, BGS, NKC, P)
        m["srcT"] = np.ascontiguousarray(blk.transpose(0, 3, 2, 1)).reshape(
            NST * P, NKC * BGS
        )
        in_maps.append(m)
    return in_maps


def _ensure_axon_hooks():
    """bass_utils imports antenv.axon_hooks when tracing; this image lacks it."""
    if "antenv.axon_hooks" in sys.modules:
        return
    import types

    mod = types.ModuleType("antenv.axon_hooks")
    mod._hook = None
    mod.set_axon_ntff_profile_hook = lambda h: setattr(mod, "_hook", h)
    mod.get_axon_ntff_profile_hook = lambda: mod._hook
    sys.modules["antenv.axon_hooks"] = mod
    try:
        from trn_agent_boot.trn_boot import _ntff_profile_via_ctypes

        mod.set_axon_ntff_profile_hook(
            _ntff_profile_via_ctypes("/opt/axon/libaxon_pjrt.so")
        )
    except Exception:
        pass


def kernel(src, attn_mask, W, b, W_rev, b_rev, **_ignored):
    _ensure_axon_hooks()
    from concourse import bass_utils

    src = np.asarray(src, dtype=np.float32)
    W = np.asarray(W, dtype=np.float32)
    b = np.asarray(b, dtype=np.float32)
    W_rev = np.asarray(W_rev, dtype=np.float32)
    b_rev = np.asarray(b_rev, dtype=np.float32)

    nc = _get_nc()
    in_maps = _build_in_maps(src, W, b, W_rev, b_rev)
    res = bass_utils.run_bass_kernel_spmd(nc, in_maps, core_ids=list(range(N_CORES)))
    out = np.concatenate([res.results[i]["out"] for i in range(N_CORES)], axis=0)
    return out.astype(np.float32)


if __name__ == "__main__":
    rng = np.random.default_rng(0)
    inputs = {
        "src": rng.standard_normal((B, D), dtype=np.float32),
        "attn_mask": np.ones((B,), np.float32),
        "W": (rng.standard_normal((C, D + C)) / 32.0).astype(np.float32),
        "b": (rng.standard_normal((C,)) / 32.0).astype(np.float32),
        "W_rev": (rng.standard_normal((C, D + C)) / 32.0).astype(np.float32),
        "b_rev": (rng.standard_normal((C,)) / 32.0).astype(np.float32),
    }
    out = kernel(**inputs)
    print("out", out.shape, out.dtype, out.min(), out.max())


# revision 17
# speedup vs baseline: 1.1026x; 1.1026x over previous
"""BiChain kernel for 8x TRN2 NeuronCores (data-parallel over batch).

Math: for each chain (fwd, rev), score_i = sigmoid(<[src, s_0..s_{i-1}], w_i> + b_i).
Split w_i into the dense part (first 1024 cols) and the tiny triangular coupling
U[i,j] = W[i, 1024+j].  Then  S = sigmoid(G + b + U S)  with  G = src @ Wd.T,
solved by one Jacobi refinement (U is nilpotent, coupling norm ~0.3):
S1 = sigmoid(G+b), S2 = sigmoid(G+b + U S1).  The rev chain is stored
row-reversed so the final combine 0.5*(S_f + S_r) is row-aligned and fused with
the transpose back to [batch, 40] as a single matmul against [0.5*I; 0.5*I].

Layout: src is cast to fp16 and TRANSPOSED on the host, so the device load is a
fully contiguous [D, batch] read (half the HBM traffic of the f32 source, no
on-chip transposes at all).  G accumulates in PSUM [80, batch-group]; the U*S1
refinement matmul accumulates onto the still-open G group, so G never leaves
PSUM.  Per-256-column pipeline: 8 G matmuls -> sigmoid -> U matmul -> sigmoid
-> fuse/transpose matmul -> store, software-pipelined across batch groups.
"""

import os
import sys

sys.path.insert(0, "/opt/trn_rl_repo")

import numpy as np

B, D, C = 32768, 1024, 40
C2 = 2 * C
N_CORES = 8
BS = B // N_CORES          # 4096 rows per core
P = 128
NKC = D // P               # 8 contraction chunks
BGS = int(os.environ.get("BICHAIN_BGS", "256"))   # batch-group (pipeline step) size
NST = BS // BGS            # pipeline steps per core
TPG = BGS // P             # output row-tiles per step
NT = BS // P               # 32 output row-tiles per core

_CACHE = {}


def _host_prep(W, b, W_rev, b_rev):
    Wr = W_rev[::-1].copy()
    br = b_rev[::-1].copy()
    Uf = np.zeros((C, C), np.float32)
    Ur = np.zeros((C, C), np.float32)
    for i in range(C):
        for j in range(C):
            if j < i:
                Uf[i, j] = W[i, D + j]
            if j > i:
                Ur[i, j] = Wr[i, D + (C - 1 - j)]
    Wd = np.concatenate([W[:, :D], Wr[:, :D]], axis=0)            # [80, 1024]
    # wt[p, kc, m] = Wd.T[kc*128 + p, m]: contiguous per-partition const load
    wt = np.ascontiguousarray(
        Wd.T.reshape(NKC, P, C2).transpose(1, 0, 2)
    ).astype(np.float16)                                          # [128, 8, 80]
    u2t = np.zeros((C2, C2), np.float32)
    u2t[:C, :C] = Uf.T
    u2t[C:, C:] = Ur.T
    u2t = u2t.astype(np.float16)
    bvec = np.concatenate([b, br]).reshape(C2, 1).astype(np.float32)
    halfi = np.zeros((C2, C), np.float32)
    halfi[np.arange(C), np.arange(C)] = 0.5
    halfi[C + np.arange(C), np.arange(C)] = 0.5
    halfi = halfi.astype(np.float16)
    return {"wt": wt.reshape(P, NKC * C2), "u2t": u2t, "bvec": bvec, "halfi": halfi}


def build_nc():
    from concourse import bacc, mybir
    from concourse.tile import TileContext

    dt = mybir.dt
    AF = mybir.ActivationFunctionType

    nc = bacc.Bacc(None, target_bir_lowering=False, debug=False)
    # srcT row g*128+p holds chunk g's data for partition p as one contiguous
    # 4KB run: [kc, n] for n in the chunk's 256 batch columns
    srcT = nc.declare_dram_parameter("srcT", [NST * P, NKC * BGS], dt.float16, isOutput=False)
    wt = nc.declare_dram_parameter("wt", [P, NKC * C2], dt.float16, isOutput=False)
    u2t = nc.declare_dram_parameter("u2t", [C2, C2], dt.float16, isOutput=False)
    bvec = nc.declare_dram_parameter("bvec", [C2, 1], dt.float32, isOutput=False)
    halfi = nc.declare_dram_parameter("halfi", [C2, C], dt.float16, isOutput=False)
    out = nc.declare_dram_parameter("out", [BS, C], dt.float32, isOutput=True)

    with TileContext(nc) as tc:
        with (
            tc.tile_pool(name="const", bufs=1) as cpool,
            tc.tile_pool(name="big", bufs=1) as bigpool,
            tc.tile_pool(name="gps", bufs=3, space="PSUM") as gpool,
            tc.tile_pool(name="ops", bufs=2, space="PSUM") as opool,
        ):
            # consts go via the sync engine's queue so they don't occupy the
            # issue slots of the src stream
            wt_sb = cpool.tile([P, NKC, C2], dt.float16)
            nc.sync.dma_start(
                out=wt_sb[:], in_=wt[:].rearrange("p (kc m) -> p kc m", m=C2)
            )
            u2t_sb = cpool.tile([C2, C2], dt.float16)
            nc.sync.dma_start(out=u2t_sb[:], in_=u2t[:])
            b_sb = cpool.tile([C2, 1], dt.float32)
            nc.sync.dma_start(out=b_sb[:], in_=bvec[:])
            halfi_sb = cpool.tile([C2, C], dt.float16)
            nc.sync.dma_start(out=halfi_sb[:], in_=halfi[:])

            # srcT_sb[p, g, kc, n] = src[g*256 + n, kc*128 + p]
            srcT_sb = bigpool.tile([P, NST, NKC, BGS], dt.float16)
            s1 = bigpool.tile([C2, BS], dt.float16)
            sfin = bigpool.tile([C2, BS], dt.float16)
            outst = bigpool.tile([64, NST, 4, C], dt.float32)

            srcT_pt = srcT[:].rearrange("(g p) x -> p g x", p=P)

            def issue_load(g):
                # gpsimd sw ring: strict FIFO, merges contiguous partition
                # rows into big descriptors (full engine rate)
                nc.gpsimd.dma_start(
                    out=srcT_sb[:, g, :, :].rearrange("p kc n -> p (kc n)"),
                    in_=srcT_pt[:, g, :],
                )

            LOOKAHEAD = 4
            for g in range(LOOKAHEAD):
                issue_load(g)

            # batch row = s*BGS + 4p + e  ->  [64, NST, 4*C] with 640B runs
            out_pt = out[:].rearrange("(s p e) c -> p s (e c)", p=64, e=4)
            gtiles = [None] * NST

            # software-pipelined steps: G(s) | jacobi(s-1) | fuse+store(s-2)
            for step in range(NST + 2):
                if step + LOOKAHEAD < NST:
                    issue_load(step + LOOKAHEAD)
                if step < NST:
                    s = step
                    sl = slice(s * BGS, (s + 1) * BGS)
                    ps = gpool.tile([C2, 512], dt.float32, name="psg")  # full psum bank
                    gtiles[s] = ps
                    for kc in range(NKC):
                        nc.tensor.matmul(
                            ps[:, :BGS],
                            lhsT=wt_sb[:, kc, :],
                            rhs=srcT_sb[:, s, kc, :],
                            start=(kc == 0),
                            stop=False,   # group stays open for the U*S1 accumulate
                        )
                    # S1 = sigmoid(G + b) straight off psum (bias is per-partition)
                    nc.scalar.activation(
                        out=s1[:, sl], in_=ps[:, :BGS], func=AF.Sigmoid, bias=b_sb[:]
                    )
                if 1 <= step <= NST:
                    s = step - 1
                    sl = slice(s * BGS, (s + 1) * BGS)
                    ps = gtiles[s]
                    # G += U @ S1 (accumulates onto the open group; WAR on the
                    # sigmoid read is handled by the tile dep tracker)
                    nc.tensor.matmul(
                        ps[:, :BGS],
                        lhsT=u2t_sb[:],
                        rhs=s1[:, sl],
                        start=False,
                        stop=True,
                    )
                    nc.scalar.activation(
                        out=sfin[:, sl], in_=ps[:, :BGS], func=AF.Sigmoid, bias=b_sb[:]
                    )
                if step >= 2:
                    s = step - 2
                    # fused 0.5*(S_f + S_r) + transpose back to [batch, 40].
                    # Partition p takes batch rows s*BGS + 4p + e, so the store
                    # writes 640B-contiguous runs per partition (fast DMA).
                    sfe = sfin[:, s * BGS : (s + 1) * BGS].rearrange(
                        "c (q e) -> c e q", e=4
                    )
                    ps_o = opool.tile([64, 4, C], dt.float32, name="pso")
                    for e in range(4):
                        nc.tensor.matmul(
                            ps_o[:, e, :],
                            lhsT=sfe[:, e, :],
                            rhs=halfi_sb[:],
                            start=(e == 0),
                            stop=(e == 3),
                        )
                    nc.vector.tensor_copy(outst[:, s, :, :], ps_o[:])
                    # stores interleave into the same sw ring between chunk
                    # issues: FIFO position is mid-stream, the DRAM span per
                    # step-pair is one contiguous 80KB block -> full rate
                    if s % 2 == 1:
                        nc.gpsimd.dma_start(
                            out=out_pt[:, s - 1 : s + 1, :],
                            in_=outst[:, s - 1 : s + 1, :, :].rearrange(
                                "p s e c -> p s (e c)"
                            ),
                        )

    nc.compile()
    return nc


def _get_nc():
    if "nc" not in _CACHE:
        _CACHE["nc"] = build_nc()
    return _CACHE["nc"]


def _build_in_maps(src, W, b, W_rev, b_rev):
    prep = _host_prep(W, b, W_rev, b_rev)
    src16 = np.asarray(src, dtype=np.float16)
    in_maps = []
    for c in range(N_CORES):
        m = dict(prep)
        # [NST*P, NKC*BGS]: row g*128+p = [kc, n] slab for chunk g, partition p
        blk = src16[c * BS : (c + 1) * BS].reshape(NST, BGS, NKC, P)
        m["srcT"] = np.ascontiguousarray(blk.transpose(0, 3, 2, 1)).reshape(
            NST * P, NKC * BGS
        )
        in_maps.append(m)
    return in_maps


def _ensure_axon_hooks():
    """bass_utils imports antenv.axon_hooks when tracing; this image lacks it."""
    if "antenv.axon_hooks" in sys.modules:
        return
    import types

    mod = types.ModuleType("antenv.axon_hooks")
    mod._hook = None
    mod.set_axon_ntff_profile_hook = lambda h: setattr(mod, "_hook", h)
    mod.get_axon_ntff_profile_hook = lambda: mod._hook
    sys.modules["antenv.axon_hooks"] = mod
    try:
        from trn_agent_boot.trn_boot import _ntff_profile_via_ctypes

        mod.set_axon_ntff_profile_hook(
            _ntff_profile_via_ctypes("/opt/axon/libaxon_pjrt.so")
        )
    except Exception:
        pass


def kernel(src, attn_mask, W, b, W_rev, b_rev, **_ignored):
    _ensure_axon_hooks()
    from concourse import bass_utils

    src = np.asarray(src, dtype=np.float32)
    W = np.asarray(W, dtype=np.float32)
    b = np.asarray(b, dtype=np.float32)
    W_rev = np.asarray(W_rev, dtype=np.float32)
    b_rev = np.asarray(b_rev, dtype=np.float32)

    nc = _get_nc()
    in_maps = _build_in_maps(src, W, b, W_rev, b_rev)
    res = bass_utils.run_bass_kernel_spmd(nc, in_maps, core_ids=list(range(N_CORES)))
    out = np.concatenate([res.results[i]["out"] for i in range(N_CORES)], axis=0)
    return out.astype(np.float32)


if __name__ == "__main__":
    rng = np.random.default_rng(0)
    inputs = {
        "src": rng.standard_normal((B, D), dtype=np.float32),
        "attn_mask": np.ones((B,), np.float32),
        "W": (rng.standard_normal((C, D + C)) / 32.0).astype(np.float32),
        "b": (rng.standard_normal((C,)) / 32.0).astype(np.float32),
        "W_rev": (rng.standard_normal((C, D + C)) / 32.0).astype(np.float32),
        "b_rev": (rng.standard_normal((C,)) / 32.0).astype(np.float32),
    }
    out = kernel(**inputs)
    print("out", out.shape, out.dtype, out.min(), out.max())


# revision 20
# speedup vs baseline: 1.2114x; 1.0987x over previous
"""BiChain kernel for 8x TRN2 NeuronCores (data-parallel over batch).

Math: for each chain (fwd, rev), score_i = sigmoid(<[src, s_0..s_{i-1}], w_i> + b_i).
Split w_i into the dense part (first 1024 cols) and the tiny triangular coupling
U[i,j] = W[i, 1024+j].  Then  S = sigmoid(G + b + U S)  with  G = src @ Wd.T,
solved by one Jacobi refinement (U is nilpotent, coupling norm ~0.3):
S1 = sigmoid(G+b), S2 = sigmoid(G+b + U S1).  The rev chain is stored
row-reversed so the final combine 0.5*(S_f + S_r) is row-aligned and fused with
the transpose back to [batch, 40] as a single matmul against [0.5*I; 0.5*I].

Layout: src is cast to fp16 and TRANSPOSED on the host, so the device load is a
fully contiguous [D, batch] read (half the HBM traffic of the f32 source, no
on-chip transposes at all).  G accumulates in PSUM [80, batch-group]; the U*S1
refinement matmul accumulates onto the still-open G group, so G never leaves
PSUM.  Per-256-column pipeline: 8 G matmuls -> sigmoid -> U matmul -> sigmoid
-> fuse/transpose matmul -> store, software-pipelined across batch groups.
"""

import os
import sys

sys.path.insert(0, "/opt/trn_rl_repo")

import numpy as np

B, D, C = 32768, 1024, 40
C2 = 2 * C
N_CORES = 8
BS = B // N_CORES          # 4096 rows per core
P = 128
NKC = D // P               # 8 contraction chunks
BGS = int(os.environ.get("BICHAIN_BGS", "256"))   # batch-group (pipeline step) size
NST = BS // BGS            # pipeline steps per core
TPG = BGS // P             # output row-tiles per step
NT = BS // P               # 32 output row-tiles per core

_CACHE = {}


def _host_prep(W, b, W_rev, b_rev):
    Wr = W_rev[::-1].copy()
    br = b_rev[::-1].copy()
    Uf = np.zeros((C, C), np.float32)
    Ur = np.zeros((C, C), np.float32)
    for i in range(C):
        for j in range(C):
            if j < i:
                Uf[i, j] = W[i, D + j]
            if j > i:
                Ur[i, j] = Wr[i, D + (C - 1 - j)]
    Wd = np.concatenate([W[:, :D], Wr[:, :D]], axis=0)            # [80, 1024]
    # wt[p, kc, m] = Wd.T[kc*128 + p, m]: contiguous per-partition const load
    wt = np.ascontiguousarray(
        Wd.T.reshape(NKC, P, C2).transpose(1, 0, 2)
    ).astype(np.float16)                                          # [128, 8, 80]
    u2t = np.zeros((C2, C2), np.float32)
    u2t[:C, :C] = Uf.T
    u2t[C:, C:] = Ur.T
    u2t = u2t.astype(np.float16)
    bvec = np.concatenate([b, br]).reshape(C2, 1).astype(np.float32)
    halfi = np.zeros((C2, C), np.float32)
    halfi[np.arange(C), np.arange(C)] = 0.5
    halfi[C + np.arange(C), np.arange(C)] = 0.5
    halfi = halfi.astype(np.float16)
    return {"wt": wt.reshape(P, NKC * C2), "u2t": u2t, "bvec": bvec, "halfi": halfi}


def build_nc():
    from concourse import bacc, mybir
    from concourse.tile import TileContext

    dt = mybir.dt
    AF = mybir.ActivationFunctionType

    nc = bacc.Bacc(None, target_bir_lowering=False, debug=False)
    # srcT row g*128+p holds chunk g's data for partition p as one contiguous
    # 4KB run: [kc, n] for n in the chunk's 256 batch columns
    srcT = nc.declare_dram_parameter("srcT", [NST * P, NKC * BGS], dt.float16, isOutput=False)
    wt = nc.declare_dram_parameter("wt", [P, NKC * C2], dt.float16, isOutput=False)
    u2t = nc.declare_dram_parameter("u2t", [C2, C2], dt.float16, isOutput=False)
    bvec = nc.declare_dram_parameter("bvec", [C2, 1], dt.float32, isOutput=False)
    halfi = nc.declare_dram_parameter("halfi", [C2, C], dt.float16, isOutput=False)
    out = nc.declare_dram_parameter("out", [BS, C], dt.float32, isOutput=True)

    with TileContext(nc) as tc:
        with (
            tc.tile_pool(name="const", bufs=1) as cpool,
            tc.tile_pool(name="big", bufs=1) as bigpool,
            tc.tile_pool(name="gps", bufs=3, space="PSUM") as gpool,
            tc.tile_pool(name="ops", bufs=2, space="PSUM") as opool,
        ):
            # consts go via the sync engine's queue so they don't occupy the
            # issue slots of the src stream
            wt_sb = cpool.tile([P, NKC, C2], dt.float16)
            nc.sync.dma_start(
                out=wt_sb[:], in_=wt[:].rearrange("p (kc m) -> p kc m", m=C2)
            )
            u2t_sb = cpool.tile([C2, C2], dt.float16)
            nc.sync.dma_start(out=u2t_sb[:], in_=u2t[:])
            b_sb = cpool.tile([C2, 1], dt.float32)
            nc.sync.dma_start(out=b_sb[:], in_=bvec[:])
            halfi_sb = cpool.tile([C2, C], dt.float16)
            nc.sync.dma_start(out=halfi_sb[:], in_=halfi[:])

            # srcT_sb[p, g, kc, n] = src[g*256 + n, kc*128 + p]
            srcT_sb = bigpool.tile([P, NST, NKC, BGS], dt.float16)
            s1 = bigpool.tile([C2, BS], dt.float16)
            sfin = bigpool.tile([C2, BS], dt.float16)
            outst = bigpool.tile([64, NST, 4, C], dt.float32)

            srcT_pt = srcT[:].rearrange("(g p) x -> p g x", p=P)

            def issue_load(g):
                # gpsimd sw ring: strict FIFO, merges contiguous partition
                # rows into big descriptors (full engine rate)
                nc.gpsimd.dma_start(
                    out=srcT_sb[:, g, :, :].rearrange("p kc n -> p (kc n)"),
                    in_=srcT_pt[:, g, :],
                )

            # all loads issued upfront: the ring must contain no waiting
            # instructions or later chunk issues stall behind them
            for g in range(NST):
                issue_load(g)

            # batch row = s*BGS + 4p + e  ->  [64, NST, 4*C] with 640B runs
            out_pt = out[:].rearrange("(s p e) c -> p s (e c)", p=64, e=4)
            gtiles = [None] * NST

            # software-pipelined steps: G(s) | jacobi(s-1) | fuse+store(s-2)
            for step in range(NST + 2):
                if step < NST:
                    s = step
                    sl = slice(s * BGS, (s + 1) * BGS)
                    ps = gpool.tile([C2, 512], dt.float32, name="psg")  # full psum bank
                    gtiles[s] = ps
                    for kc in range(NKC):
                        nc.tensor.matmul(
                            ps[:, :BGS],
                            lhsT=wt_sb[:, kc, :],
                            rhs=srcT_sb[:, s, kc, :],
                            start=(kc == 0),
                            stop=False,   # group stays open for the U*S1 accumulate
                        )
                    # S1 = sigmoid(G + b) straight off psum (bias is per-partition)
                    nc.scalar.activation(
                        out=s1[:, sl], in_=ps[:, :BGS], func=AF.Sigmoid, bias=b_sb[:]
                    )
                if 1 <= step <= NST:
                    s = step - 1
                    sl = slice(s * BGS, (s + 1) * BGS)
                    ps = gtiles[s]
                    # G += U @ S1 (accumulates onto the open group; WAR on the
                    # sigmoid read is handled by the tile dep tracker)
                    nc.tensor.matmul(
                        ps[:, :BGS],
                        lhsT=u2t_sb[:],
                        rhs=s1[:, sl],
                        start=False,
                        stop=True,
                    )
                    nc.scalar.activation(
                        out=sfin[:, sl], in_=ps[:, :BGS], func=AF.Sigmoid, bias=b_sb[:]
                    )
                if step >= 2:
                    s = step - 2
                    # fused 0.5*(S_f + S_r) + transpose back to [batch, 40].
                    # Partition p takes batch rows s*BGS + 4p + e, so the store
                    # writes 640B-contiguous runs per partition (fast DMA).
                    sfe = sfin[:, s * BGS : (s + 1) * BGS].rearrange(
                        "c (q e) -> c e q", e=4
                    )
                    ps_o = opool.tile([64, 4, C], dt.float32, name="pso")
                    for e in range(4):
                        nc.tensor.matmul(
                            ps_o[:, e, :],
                            lhsT=sfe[:, e, :],
                            rhs=halfi_sb[:],
                            start=(e == 0),
                            stop=(e == 3),
                        )
                    nc.vector.tensor_copy(outst[:, s, :, :], ps_o[:])
                    # per-step store on SP's hw queue: small (40KB), overlaps
                    # the load, and its copy-wait can't block the load ring
                    nc.sync.dma_start(
                        out=out_pt[:, s, :],
                        in_=outst[:, s, :, :].rearrange("p e c -> p (e c)"),
                    )

    nc.compile()
    return nc


def _get_nc():
    if "nc" not in _CACHE:
        _CACHE["nc"] = build_nc()
    return _CACHE["nc"]


def _build_in_maps(src, W, b, W_rev, b_rev):
    prep = _host_prep(W, b, W_rev, b_rev)
    src16 = np.asarray(src, dtype=np.float16)
    in_maps = []
    for c in range(N_CORES):
        m = dict(prep)
        # [NST*P, NKC*BGS]: row g*128+p = [kc, n] slab for chunk g, partition p
        blk = src16[c * BS : (c + 1) * BS].reshape(NST, BGS, NKC, P)
        m["srcT"] = np.ascontiguousarray(blk.transpose(0, 3, 2, 1)).reshape(
            NST * P, NKC * BGS
        )
        in_maps.append(m)
    return in_maps


def _ensure_axon_hooks():
    """bass_utils imports antenv.axon_hooks when tracing; this image lacks it."""
    if "antenv.axon_hooks" in sys.modules:
        return
    import types

    mod = types.ModuleType("antenv.axon_hooks")
    mod._hook = None
    mod.set_axon_ntff_profile_hook = lambda h: setattr(mod, "_hook", h)
    mod.get_axon_ntff_profile_hook = lambda: mod._hook
    sys.modules["antenv.axon_hooks"] = mod
    try:
        from trn_agent_boot.trn_boot import _ntff_profile_via_ctypes

        mod.set_axon_ntff_profile_hook(
            _ntff_profile_via_ctypes("/opt/axon/libaxon_pjrt.so")
        )
    except Exception:
        pass


def kernel(src, attn_mask, W, b, W_rev, b_rev, **_ignored):
    _ensure_axon_hooks()
    from concourse import bass_utils

    src = np.asarray(src, dtype=np.float32)
    W = np.asarray(W, dtype=np.float32)
    b = np.asarray(b, dtype=np.float32)
    W_rev = np.asarray(W_rev, dtype=np.float32)
    b_rev = np.asarray(b_rev, dtype=np.float32)

    nc = _get_nc()
    in_maps = _build_in_maps(src, W, b, W_rev, b_rev)
    res = bass_utils.run_bass_kernel_spmd(nc, in_maps, core_ids=list(range(N_CORES)))
    out = np.concatenate([res.results[i]["out"] for i in range(N_CORES)], axis=0)
    return out.astype(np.float32)


if __name__ == "__main__":
    rng = np.random.default_rng(0)
    inputs = {
        "src": rng.standard_normal((B, D), dtype=np.float32),
        "attn_mask": np.ones((B,), np.float32),
        "W": (rng.standard_normal((C, D + C)) / 32.0).astype(np.float32),
        "b": (rng.standard_normal((C,)) / 32.0).astype(np.float32),
        "W_rev": (rng.standard_normal((C, D + C)) / 32.0).astype(np.float32),
        "b_rev": (rng.standard_normal((C,)) / 32.0).astype(np.float32),
    }
    out = kernel(**inputs)
    print("out", out.shape, out.dtype, out.min(), out.max())
